# revision 1
# baseline (speedup 1.0000x reference)
"""Trainium2 Bass kernel for nn_CustomLoss (cross-entropy + worst-class masked loss).

Computes: loss = mean_i(logsumexp(output_i) - output_i[target_i])
          result = loss * (1 + mean_i(target_i in {3,5,8,9}))

Data-parallel over 8 NeuronCores: each core streams its 32768x1000 f32 shard,
computing per-row logsumexp (ACT engine: exp with free-dim accumulation, then
one Ln pass) and the target logit (DVE: fused (iota==t)*x row-reduce), plus the
worst-class membership count. Host combines the per-core partial sums.
"""
import numpy as np
from contextlib import ExitStack

import concourse.bacc as bacc
import concourse.tile as tile
from concourse import mybir
from concourse.bass_utils import run_bass_kernel_spmd

F32 = mybir.dt.float32
AF = mybir.ActivationFunctionType
ALU = mybir.AluOpType

N_CORES = 8
B, C = 262144, 1000
ROWS = B // N_CORES           # 32768 rows per core
P = 128                       # SBUF partitions
G = 4                         # [128, C] sub-tiles per DMA chunk
N_CHUNKS = ROWS // (P * G)    # 64 chunks of [128, G*C] (2 MB contiguous)
N_TILES = ROWS // P           # 256 logical [128, C] tiles
WORST = (3.0, 5.0, 8.0, 9.0)

_CACHE = {}


def _build(bufs_x: int = 4):
    return _build_T(1, bufs_x)


def _build_T(T: int, bufs_x: int = 4):
    nc = bacc.Bacc(None, target_bir_lowering=False, debug=False,
                   num_devices=N_CORES)
    x_h = nc.declare_dram_parameter("x", [N_CHUNKS, P, G * C], F32, isOutput=False)
    tgt_h = nc.declare_dram_parameter("tgt", [P, N_TILES], F32, isOutput=False)
    iota_h = nc.declare_dram_parameter("iota", [P, C], F32, isOutput=False)
    out_h = nc.declare_dram_parameter("out", [P, 2], F32, isOutput=True)

    with tile.TileContext(nc) as tc, ExitStack() as ctx:
        xp = ctx.enter_context(tc.tile_pool(name="xp", bufs=bufs_x))
        scr = ctx.enter_context(tc.tile_pool(name="scr", bufs=2))
        pers = ctx.enter_context(tc.tile_pool(name="pers", bufs=1))

        s_cols = pers.tile([P, N_TILES], F32, tag="s_cols")   # sum_j exp(x_ij)
        g_cols = pers.tile([P, N_TILES], F32, tag="g_cols")   # x_i[t_i]
        tgt_sb = pers.tile([P, N_TILES], F32, tag="tgt_sb")
        iota_sb = pers.tile([P, C], F32, tag="iota_sb")
        fin = pers.tile([P, 8], F32, tag="fin")
        out_sb = pers.tile([P, 2], F32, tag="out_sb")

        nc.sync.dma_start(out=tgt_sb[:], in_=tgt_h[:])
        nc.sync.dma_start(out=iota_sb[:], in_=iota_h[:])

        for _rep in range(T):
            _body_once(nc, tc, ctx, xp, scr, pers, x_h, out_h,
                       s_cols, g_cols, tgt_sb, iota_sb, fin, out_sb)

    nc.compile()
    return nc


def _body_once(nc, tc, ctx, xp, scr, pers, x_h, out_h,
               s_cols, g_cols, tgt_sb, iota_sb, fin, out_sb):
    if True:
        for ch in range(N_CHUNKS):
            x_t = xp.tile([P, G * C], F32, tag="x_t")
            nc.sync.dma_start(out=x_t[:], in_=x_h[ch])
            for j in range(G):
                k = ch * G + j
                xs = x_t[:, j * C:(j + 1) * C]
                e_scr = scr.tile([P, C], F32, tag="e_scr")
                m_scr = scr.tile([P, C], F32, tag="m_scr")
                # s_cols[p,k] = sum_j exp(x[p,j])
                nc.scalar.activation(
                    out=e_scr[:], in_=xs, func=AF.Exp,
                    accum_out=s_cols[:, k:k + 1],
                )
                # g_cols[p,k] = sum_j (iota==t) * x = x[p, t_p]
                nc.vector.scalar_tensor_tensor(
                    out=m_scr[:], in0=iota_sb[:], scalar=tgt_sb[:, k:k + 1],
                    in1=xs, op0=ALU.is_equal, op1=ALU.mult,
                    accum_out=g_cols[:, k:k + 1],
                )

        # fin0 = sum_k ln(s_k); fin1 = sum_k x_t,k
        lse_cols = pers.tile([P, N_TILES], F32, tag="lse_cols")
        nc.scalar.activation(
            out=lse_cols[:], in_=s_cols[:], func=AF.Ln,
            accum_out=fin[:, 0:1],
        )
        nc.vector.tensor_reduce(
            out=fin[:, 1:2], in_=g_cols[:], axis=mybir.AxisListType.X, op=ALU.add,
        )
        nc.vector.tensor_tensor(
            out=out_sb[:, 0:1], in0=fin[:, 0:1], in1=fin[:, 1:2], op=ALU.subtract,
        )
        # out col1 = count of targets in WORST classes
        eq = pers.tile([P, N_TILES], F32, tag="eq")
        nc.vector.tensor_scalar(
            out=eq[:], in0=tgt_sb[:], scalar1=WORST[0], scalar2=None,
            op0=ALU.is_equal,
        )
        for v in WORST[1:-1]:
            nc.vector.scalar_tensor_tensor(
                out=eq[:], in0=tgt_sb[:], scalar=v, in1=eq[:],
                op0=ALU.is_equal, op1=ALU.add,
            )
        nc.vector.scalar_tensor_tensor(
            out=eq[:], in0=tgt_sb[:], scalar=WORST[-1], in1=eq[:],
            op0=ALU.is_equal, op1=ALU.add,
            accum_out=out_sb[:, 1:2],
        )

        nc.sync.dma_start(out=out_h[:], in_=out_sb[:])


def _shard_inputs(output: np.ndarray, target: np.ndarray):
    iota = np.tile(np.arange(C, dtype=np.float32), (P, 1))
    in_maps = []
    for c in range(N_CORES):
        xs = output[c * ROWS:(c + 1) * ROWS]
        ts = target[c * ROWS:(c + 1) * ROWS].astype(np.float32)
        # tgt[p, G*g+j] = target[c*ROWS + (P*G)*g + G*p + j]
        tgt = ts.reshape(N_CHUNKS, P, G).transpose(1, 0, 2).reshape(P, N_TILES)
        in_maps.append({
            "x": np.ascontiguousarray(xs.reshape(N_CHUNKS, P, G * C)),
            "tgt": np.ascontiguousarray(tgt),
            "iota": iota,
        })
    return in_maps


def _combine(results) -> np.float32:
    nll = 0.0
    cnt = 0.0
    for r in results:
        nll += float(r["out"][:, 0].astype(np.float64).sum())
        cnt += float(r["out"][:, 1].astype(np.float64).sum())
    loss = nll / B
    mask_mean = cnt / B
    return np.float32(loss * (1.0 + mask_mean))


def _run(in_maps, **kwargs):
    if "nc" not in _CACHE:
        _CACHE["nc"] = _build()
    return run_bass_kernel_spmd(_CACHE["nc"], in_maps, list(range(N_CORES)),
                                **kwargs)


def kernel(output: np.ndarray, target: np.ndarray) -> np.float32:
    assert output.shape == (B, C) and target.shape == (B,)
    res = _run(_shard_inputs(output, target))
    return _combine(res.results)



# revision 2
# speedup vs baseline: 1.0788x; 1.0788x over previous
"""Trainium2 Bass kernel v3 for nn_CustomLoss (cross-entropy + worst-class masked loss).

Computes: loss = mean_i(logsumexp(output_i) - output_i[target_i])
          result = loss * (1 + mean_i(target_i in {3,5,8,9}))

Data-parallel over 8 NeuronCores: each core streams its 32768x1000 f32 shard
in 32 chunks of 4 MB ([128, 8x1000]) — measured 417 GB/s vs 330 GB/s for 2 MB
chunks. Per [128,1000] tile: ACT computes exp with free-dim accumulation
(per-row sum for logsumexp), DVE computes the target logit via a fused
(iota==t)*x row-reduce. The two engines read the chunk independently (exp
output goes to a dead scratch) so neither serializes behind the other.
Host combines the per-core partial sums.
"""
import numpy as np
from contextlib import ExitStack

import concourse.bacc as bacc
import concourse.tile as tile
from concourse import mybir
from concourse.bass_utils import run_bass_kernel_spmd

F32 = mybir.dt.float32
AF = mybir.ActivationFunctionType
ALU = mybir.AluOpType

N_CORES = 8
B, C = 262144, 1000
ROWS = B // N_CORES           # 32768 rows per core
P = 128                       # SBUF partitions
G = 8                         # [128, C] sub-tiles per DMA chunk
N_CHUNKS = ROWS // (P * G)    # 32 chunks of [128, G*C] (4 MB contiguous)
N_TILES = ROWS // P           # 256 logical [128, C] tiles
WORST = (3.0, 5.0, 8.0, 9.0)

_CACHE = {}


def _build(bufs_x: int = 4):
    nc = bacc.Bacc(None, target_bir_lowering=False, debug=False,
                   num_devices=N_CORES)
    x_h = nc.declare_dram_parameter("x", [N_CHUNKS, P, G * C], F32, isOutput=False)
    tgt_h = nc.declare_dram_parameter("tgt", [P, N_TILES], F32, isOutput=False)
    iota_h = nc.declare_dram_parameter("iota", [P, C], F32, isOutput=False)
    out_h = nc.declare_dram_parameter("out", [P, 2], F32, isOutput=True)

    with tile.TileContext(nc) as tc, ExitStack() as ctx:
        xp = ctx.enter_context(tc.tile_pool(name="xp", bufs=bufs_x))
        scr = ctx.enter_context(tc.tile_pool(name="scr", bufs=2))
        pers = ctx.enter_context(tc.tile_pool(name="pers", bufs=1))

        s_cols = pers.tile([P, N_TILES], F32, tag="s_cols")   # sum_j exp(x_ij)
        g_cols = pers.tile([P, N_TILES], F32, tag="g_cols")   # x_i[t_i]
        tgt_sb = pers.tile([P, N_TILES], F32, tag="tgt_sb")
        iota_sb = pers.tile([P, C], F32, tag="iota_sb")
        fin = pers.tile([P, 8], F32, tag="fin")
        out_sb = pers.tile([P, 2], F32, tag="out_sb")

        nc.sync.dma_start(out=tgt_sb[:], in_=tgt_h[:])
        nc.sync.dma_start(out=iota_sb[:], in_=iota_h[:])

        for ch in range(N_CHUNKS):
            x_t = xp.tile([P, G * C], F32, tag="x_t")
            nc.sync.dma_start(out=x_t[:], in_=x_h[ch])
            for j in range(G):
                k = ch * G + j
                xs = x_t[:, j * C:(j + 1) * C]
                e_scr = scr.tile([P, C], F32, tag="e_scr")
                m_scr = scr.tile([P, C], F32, tag="m_scr")
                # s_cols[p,k] = sum_j exp(x[p,j])
                nc.scalar.activation(
                    out=e_scr[:], in_=xs, func=AF.Exp,
                    accum_out=s_cols[:, k:k + 1],
                )
                # g_cols[p,k] = sum_j (iota==t) * x = x[p, t_p]
                nc.vector.scalar_tensor_tensor(
                    out=m_scr[:], in0=iota_sb[:], scalar=tgt_sb[:, k:k + 1],
                    in1=xs, op0=ALU.is_equal, op1=ALU.mult,
                    accum_out=g_cols[:, k:k + 1],
                )

        # fin0 = sum_k ln(s_k); fin1 = sum_k x_t,k
        lse_cols = pers.tile([P, N_TILES], F32, tag="lse_cols")
        nc.scalar.activation(
            out=lse_cols[:], in_=s_cols[:], func=AF.Ln,
            accum_out=fin[:, 0:1],
        )
        nc.vector.tensor_reduce(
            out=fin[:, 1:2], in_=g_cols[:], axis=mybir.AxisListType.X, op=ALU.add,
        )
        nc.vector.tensor_tensor(
            out=out_sb[:, 0:1], in0=fin[:, 0:1], in1=fin[:, 1:2], op=ALU.subtract,
        )
        # out col1 = count of targets in WORST classes
        eq = pers.tile([P, N_TILES], F32, tag="eq")
        nc.vector.tensor_scalar(
            out=eq[:], in0=tgt_sb[:], scalar1=WORST[0], scalar2=None,
            op0=ALU.is_equal,
        )
        for v in WORST[1:-1]:
            nc.vector.scalar_tensor_tensor(
                out=eq[:], in0=tgt_sb[:], scalar=v, in1=eq[:],
                op0=ALU.is_equal, op1=ALU.add,
            )
        nc.vector.scalar_tensor_tensor(
            out=eq[:], in0=tgt_sb[:], scalar=WORST[-1], in1=eq[:],
            op0=ALU.is_equal, op1=ALU.add,
            accum_out=out_sb[:, 1:2],
        )

        nc.sync.dma_start(out=out_h[:], in_=out_sb[:])

    nc.compile()
    return nc


def _shard_inputs(output: np.ndarray, target: np.ndarray):
    iota = np.tile(np.arange(C, dtype=np.float32), (P, 1))
    in_maps = []
    for c in range(N_CORES):
        xs = output[c * ROWS:(c + 1) * ROWS]
        ts = target[c * ROWS:(c + 1) * ROWS].astype(np.float32)
        # tgt[p, G*ch+j] = target[c*ROWS + (P*G)*ch + G*p + j]
        tgt = ts.reshape(N_CHUNKS, P, G).transpose(1, 0, 2).reshape(P, N_TILES)
        in_maps.append({
            "x": np.ascontiguousarray(xs.reshape(N_CHUNKS, P, G * C)),
            "tgt": np.ascontiguousarray(tgt),
            "iota": iota,
        })
    return in_maps


def _combine(results) -> np.float32:
    nll = 0.0
    cnt = 0.0
    for r in results:
        nll += float(r["out"][:, 0].astype(np.float64).sum())
        cnt += float(r["out"][:, 1].astype(np.float64).sum())
    loss = nll / B
    mask_mean = cnt / B
    return np.float32(loss * (1.0 + mask_mean))


def _run(in_maps, **kwargs):
    if "nc" not in _CACHE:
        _CACHE["nc"] = _build()
    return run_bass_kernel_spmd(_CACHE["nc"], in_maps, list(range(N_CORES)),
                                **kwargs)


def kernel(output: np.ndarray, target: np.ndarray) -> np.float32:
    assert output.shape == (B, C) and target.shape == (B,)
    res = _run(_shard_inputs(output, target))
    return _combine(res.results)
